# revision 8
# baseline (speedup 1.0000x reference)
"""Trainium2 Bass kernel for a 4-layer stacked-LSTM encoder (bidirectional first
layer), B=64, T=512, 512 units, vocab 32000.

Strategy: pure SPMD data-parallel over batch (8 rows/core). Per core:
  phase A : embedding gather + x@W precompute for layer1 fwd/bwd (bwd stored
            time-reversed so its recurrence reads forward)
  phase L1: fwd and bwd recurrences interleaved (independent streams fill each
            other's pipeline gaps)
  phase C2/L2/C3/L3/C4/L4: x@W precompute then recurrence for layers 2..4

Recurrent step layout: gates f,i,o live at partition offsets 0/32/64 of one
PSUM tile (PE matmul outputs must start at partition 0/32/64) so one sigmoid
activation covers all three; gate g uses a second tile. The precomputed x@W
and bias enter PSUM via small "spread matrix" matmuls instead of vector adds.
h is re-transposed on the PE each step to serve as the next step's stationary
operand.
"""

import numpy as np
from contextlib import ExitStack

import concourse.mybir as mybir
from concourse import bacc, bass, tile
from concourse.bass import IndirectOffsetOnAxis
from concourse.bass_utils import run_bass_kernel_spmd

F32 = mybir.dt.float32
I32 = mybir.dt.int32
AF = mybir.ActivationFunctionType

VOCAB, EMB, UNITS = 32000, 512, 512
B, T = 64, 512
NCORES = 8
BL = B // NCORES          # batch rows per core
TB = 32                   # recurrence steps per For_i body (must be even)
PCB = 16                  # precompute rows-block = 16 timesteps x 8 batch = 128 rows


def _lstm_pass(nc, tc, ctx, streams):
    """Emit one (or two interleaved) recurrent LSTM passes.

    streams: list of dicts with keys
      u    : DRAM AP [128, 8192] recurrent weights (k-major, gate cols f,i,o,g)
      xw   : DRAM AP [T+TB, 4, 8, 512] precomputed x@W+bias (step-ordered)
      out  : DRAM AP for h output, already step-indexed view [T, 8, 512]
      cout : DRAM AP [8, 512] for final c state (or None)
    """
    sp_fio = streams[0]["sp_fio"]  # [32, 72] sbuf
    sp_g = streams[0]["sp_g"]      # [32, 8] sbuf
    ident = streams[0]["ident"]    # [128, 128] sbuf

    st = ctx.enter_context(tc.tile_pool(name="lstm_state", bufs=1))
    ps = ctx.enter_context(tc.tile_pool(name="lstm_psum", bufs=1, space="PSUM"))
    wk = ctx.enter_context(tc.tile_pool(name="lstm_work", bufs=2))
    io = ctx.enter_context(tc.tile_pool(name="lstm_io", bufs=4))

    for s in streams:
        u_sb = st.tile([128, 8192], F32, tag=f"u{s['tag']}")
        nc.sync.dma_start(out=u_sb[:], in_=s["u"])
        s["u_sb"] = u_sb
        cc = [st.tile([8, 512], F32, tag=f"c{s['tag']}{p}", name=f"c{s['tag']}{p}")
              for p in range(2)]
        hT = [st.tile([128, 32], F32, tag=f"h{s['tag']}{p}", name=f"h{s['tag']}{p}")
              for p in range(2)]
        nc.vector.memset(cc[0][:], 0.0)
        nc.vector.memset(hT[0][:], 0.0)
        s["c"] = cc
        s["hT"] = hT
        s["zA"] = ps.tile([72, 512], F32, tag=f"zA{s['tag']}", name=f"zA{s['tag']}")
        s["zB"] = ps.tile([8, 512], F32, tag=f"zB{s['tag']}", name=f"zB{s['tag']}")
        s["hT_ps"] = ps.tile([128, 32], F32, tag=f"hp{s['tag']}", name=f"hp{s['tag']}")

    def step(s, iv, t_l):
        pin = t_l % 2
        u_sb, zA, zB = s["u_sb"], s["zA"], s["zB"]
        xw_step = io.tile([32, 512], F32, tag=f"xw{s['tag']}")
        nc.sync.dma_start(
            out=xw_step[:],
            in_=s["xw"][bass.ds(iv + t_l, 1)],
        )
        # z = xw (+bias) spread into PSUM, then accumulate h @ U
        nc.tensor.matmul(out=zA[:], lhsT=sp_fio[:], rhs=xw_step[:],
                         start=True, stop=False)
        nc.tensor.matmul(out=zB[:], lhsT=sp_g[:], rhs=xw_step[:],
                         start=True, stop=False)
        hT_prev = s["hT"][pin]
        for k in range(4):
            lhsT = hT_prev[:, 8 * k:8 * (k + 1)]
            for g in range(3):  # f, i, o -> zA at partition 32*g
                nc.tensor.matmul(
                    out=zA[32 * g:32 * g + 8, :], lhsT=lhsT,
                    rhs=u_sb[:, 2048 * k + 512 * g:2048 * k + 512 * (g + 1)],
                    start=False, stop=(k == 3))
            nc.tensor.matmul(
                out=zB[:], lhsT=lhsT,
                rhs=u_sb[:, 2048 * k + 1536:2048 * (k + 1)],
                start=False, stop=(k == 3))
        # DVE tensor_tensor requires equal input base partitions; park each
        # intermediate at the partition offset of the sig-slice it pairs with.
        sig = wk.tile([72, 512], F32, tag=f"sig{s['tag']}")
        nc.scalar.activation(out=sig[:], in_=zA[:], func=AF.Sigmoid)
        tg = wk.tile([40, 512], F32, tag=f"tg{s['tag']}")
        nc.scalar.activation(out=tg[32:40, :], in_=zB[:], func=AF.Tanh)
        m1 = wk.tile([40, 512], F32, tag=f"m1{s['tag']}")
        nc.vector.tensor_mul(out=m1[32:40, :], in0=sig[0:8, :], in1=s["c"][pin][:])
        m2 = wk.tile([40, 512], F32, tag=f"m2{s['tag']}")
        nc.vector.tensor_mul(out=m2[32:40, :], in0=sig[32:40, :], in1=tg[32:40, :])
        c_new = s["c"][1 - pin]
        nc.vector.tensor_add(out=c_new[:], in0=m1[32:40, :], in1=m2[32:40, :])
        tc_ = wk.tile([72, 512], F32, tag=f"tc{s['tag']}")
        nc.scalar.activation(out=tc_[64:72, :], in_=c_new[:], func=AF.Tanh)
        h_step = io.tile([8, 512], F32, tag=f"h{s['tag']}")
        nc.vector.tensor_mul(out=h_step[:], in0=sig[64:72, :], in1=tc_[64:72, :])
        # transpose h for next step's stationary operand
        hT_ps = s["hT_ps"]
        for k in range(4):
            nc.tensor.transpose(out=hT_ps[:, 8 * k:8 * (k + 1)],
                                in_=h_step[:, 128 * k:128 * (k + 1)],
                                identity=ident[0:8, 0:8])
        nc.vector.tensor_copy(out=s["hT"][1 - pin][:], in_=hT_ps[:])
        nc.sync.dma_start(
            out=s["out"][bass.ds(iv + t_l, 1)],
            in_=h_step[:])

    with tc.For_i(0, T, TB) as iv:
        for t_l in range(TB):
            for s in streams:
                step(s, iv, t_l)

    for s in streams:
        if s["cout"] is not None:
            # final write of step T-1 lands in c[(T) % 2] == c[0] (T even)
            nc.sync.dma_start(out=s["cout"], in_=s["c"][T % 2][:])


def _precompute_pass(nc, tc, ctx, srcs, dsts, ident, ones1, gather=None):
    """xw[t] = sum_i rows_i[t] @ W_i + bias, blocked 16 timesteps at a time.

    srcs: list of (rows_dram_view [T,8,512] | None-if-gather, w_sb [128, K*2048], nk)
    dsts: list of (dst view [T,4,8,512] step-indexed, bias_sb [1,2048]) per weight-set
        -- each src contributes to every dst with its own weight tile.
    gather: (emb_ap, idx_view [T,8,1]) when the source rows come from the
        embedding table.
    """
    sb = ctx.enter_context(tc.tile_pool(name="pre_sb", bufs=2))
    ps = ctx.enter_context(tc.tile_pool(name="pre_ps", bufs=2, space="PSUM"))

    with tc.For_i(0, T, PCB) as iv:
        xts = []
        for si, (rows_view, _, _) in enumerate(srcs):
            rows = sb.tile([128, 512], F32, tag=f"rows{si}")
            if rows_view is None:
                emb_ap, idx_view = gather
                idx_t = sb.tile([128, 1], I32, tag="idx")
                nc.sync.dma_start(
                    out=idx_t[:],
                    in_=idx_view[bass.ds(iv, PCB)].rearrange("t b one -> b t one"))
                nc.gpsimd.indirect_dma_start(
                    out=rows[:], out_offset=None, in_=emb_ap,
                    in_offset=IndirectOffsetOnAxis(ap=idx_t[:, :1], axis=0))
            else:
                nc.sync.dma_start(
                    out=rows[:],
                    in_=rows_view[bass.ds(iv, PCB)].rearrange("t b u -> b t u"))
            xt_ps = ps.tile([128, 512], F32, tag=f"xt_ps{si}")
            for k in range(4):
                nc.tensor.transpose(out=xt_ps[:, 128 * k:128 * (k + 1)],
                                    in_=rows[:, 128 * k:128 * (k + 1)],
                                    identity=ident[:, :])
            xt = sb.tile([128, 512], F32, tag=f"xt{si}")
            nc.vector.tensor_copy(out=xt[:], in_=xt_ps[:])
            xts.append(xt)

        for di, (dst_view, bias_sb, w_sbs) in enumerate(dsts):
            for n in range(4):
                acc = ps.tile([128, 512], F32, tag=f"acc{di}")
                first = True
                for si, xt in enumerate(xts):
                    w_sb = w_sbs[si]
                    for k in range(4):
                        nc.tensor.matmul(
                            out=acc[:], lhsT=xt[:, 128 * k:128 * (k + 1)],
                            rhs=w_sb[:, 2048 * k + 512 * n:2048 * k + 512 * (n + 1)],
                            start=first, stop=False)
                        first = False
                nc.tensor.matmul(
                    out=acc[:], lhsT=ones1[:, :],
                    rhs=bias_sb[0:1, 512 * n:512 * (n + 1)],
                    start=False, stop=True)
                ot = sb.tile([128, 512], F32, tag=f"ot{di}")
                nc.vector.tensor_copy(out=ot[:], in_=acc[:])
                nc.sync.dma_start(
                    out=dst_view[bass.ds(iv, PCB)][:, n].rearrange("t b u -> b t u"),
                    in_=ot[:])


def _build(T_=T):
    global T
    T = T_
    nc = bacc.Bacc("TRN2", target_bir_lowering=False, debug=False,
                   num_devices=NCORES)

    def din(name, shape, dt=F32):
        return nc.dram_tensor(name, shape, dt, kind="ExternalInput").ap()

    def dout(name, shape, dt=F32):
        return nc.dram_tensor(name, shape, dt, kind="ExternalOutput").ap()

    def dtmp(name, shape, dt=F32):
        return nc.dram_tensor(name, shape, dt).ap()

    idx = din("idx", [T, BL, 1], I32)
    emb = din("emb", [VOCAB, EMB])
    ident_d = din("ident", [128, 128])
    sp_fio_d = din("sp_fio", [32, 72])
    sp_g_d = din("sp_g", [32, 8])
    ones1_d = din("ones1", [1, 128])
    w1f = din("w1f", [128, 8192]);  w1b = din("w1b", [128, 8192])
    w2f = din("w2f", [128, 8192]);  w2b = din("w2b", [128, 8192])
    w3 = din("w3", [128, 8192]);    w4 = din("w4", [128, 8192])
    u1f = din("u1f", [128, 8192]);  u1b = din("u1b", [128, 8192])
    u2 = din("u2", [128, 8192]);    u3 = din("u3", [128, 8192])
    u4 = din("u4", [128, 8192])
    b1f = din("b1f", [1, 2048]);    b1b = din("b1b", [1, 2048])
    b2 = din("b2", [1, 2048]);      b3 = din("b3", [1, 2048])
    b4 = din("b4", [1, 2048])

    xw1f = dtmp("xw1f", [T + TB, 4, BL, 512])
    xw1b = dtmp("xw1b", [T + TB, 4, BL, 512])
    xw2 = dtmp("xw2", [T + TB, 4, BL, 512])
    xw3 = dtmp("xw3", [T + TB, 4, BL, 512])
    xw4 = dtmp("xw4", [T + TB, 4, BL, 512])
    o1f = dtmp("o1f", [T, BL, 512])
    o1b = dtmp("o1b", [T, BL, 512])
    o2 = dtmp("o2", [T, BL, 512])
    o3 = dtmp("o3", [T, BL, 512])
    out4 = dout("out4", [T, BL, 512])
    c4 = dout("c4", [BL, 512])

    def load_consts(pool):
        ident = pool.tile([128, 128], F32, tag="ident")
        nc.sync.dma_start(out=ident[:], in_=ident_d)
        sp_fio = pool.tile([32, 72], F32, tag="sp_fio")
        nc.sync.dma_start(out=sp_fio[:], in_=sp_fio_d)
        sp_g = pool.tile([32, 8], F32, tag="sp_g")
        nc.sync.dma_start(out=sp_g[:], in_=sp_g_d)
        ones1 = pool.tile([1, 128], F32, tag="ones1")
        nc.sync.dma_start(out=ones1[:], in_=ones1_d)
        return ident, sp_fio, sp_g, ones1

    def load_w(pool, d, tag):
        w_sb = pool.tile([128, 8192], F32, tag=tag)
        nc.sync.dma_start(out=w_sb[:], in_=d)
        return w_sb

    # ---- phase A: gather + xw1f/xw1b ----
    with tile.TileContext(nc) as tc:
        with ExitStack() as ctx:
            cp = ctx.enter_context(tc.tile_pool(name="consts", bufs=1))
            ident, _, _, ones1 = load_consts(cp)
            wf_sb = load_w(cp, w1f, "w1f")
            wb_sb = load_w(cp, w1b, "w1b")
            bf_sb = cp.tile([1, 2048], F32, tag="b1f")
            nc.sync.dma_start(out=bf_sb[:], in_=b1f)
            bb_sb = cp.tile([1, 2048], F32, tag="b1b")
            nc.sync.dma_start(out=bb_sb[:], in_=b1b)
            _precompute_pass(
                nc, tc, ctx,
                srcs=[(None, None, 4)],
                dsts=[(xw1f[0:T], bf_sb, [wf_sb]),
                      (xw1b[0:T][::-1], bb_sb, [wb_sb])],
                ident=ident, ones1=ones1, gather=(emb, idx))

    # ---- phase L1: fwd+bwd interleaved ----
    with tile.TileContext(nc) as tc:
        with ExitStack() as ctx:
            cp = ctx.enter_context(tc.tile_pool(name="consts", bufs=1))
            ident, sp_fio, sp_g, _ = load_consts(cp)
            common = dict(sp_fio=sp_fio, sp_g=sp_g, ident=ident, cout=None)
            _lstm_pass(nc, tc, ctx, [
                dict(common, tag="f", u=u1f, xw=xw1f, out=o1f[:]),
                dict(common, tag="b", u=u1b, xw=xw1b, out=o1b[:][::-1]),
            ])

    # ---- phase C2 ----
    with tile.TileContext(nc) as tc:
        with ExitStack() as ctx:
            cp = ctx.enter_context(tc.tile_pool(name="consts", bufs=1))
            ident, _, _, ones1 = load_consts(cp)
            w2f_sb = load_w(cp, w2f, "w2f")
            w2b_sb = load_w(cp, w2b, "w2b")
            b2_sb = cp.tile([1, 2048], F32, tag="b2")
            nc.sync.dma_start(out=b2_sb[:], in_=b2)
            _precompute_pass(
                nc, tc, ctx,
                srcs=[(o1f[:], None, 4), (o1b[:], None, 4)],
                dsts=[(xw2[0:T], b2_sb, [w2f_sb, w2b_sb])],
                ident=ident, ones1=ones1)

    # ---- L2 / C3 / L3 / C4 / L4 ----
    for (u_d, xw_d, o_prev, o_out, w_d, b_d, xw_next, cout) in [
        (u2, xw2, o2, o2, w3, b3, xw3, None),
        (u3, xw3, o3, o3, w4, b4, xw4, None),
        (u4, xw4, None, out4, None, None, None, c4),
    ]:
        with tile.TileContext(nc) as tc:
            with ExitStack() as ctx:
                cp = ctx.enter_context(tc.tile_pool(name="consts", bufs=1))
                ident, sp_fio, sp_g, _ = load_consts(cp)
                _lstm_pass(nc, tc, ctx, [
                    dict(tag="s", sp_fio=sp_fio, sp_g=sp_g, ident=ident,
                         u=u_d, xw=xw_d, out=o_out[:], cout=cout),
                ])
        if w_d is not None:
            with tile.TileContext(nc) as tc:
                with ExitStack() as ctx:
                    cp = ctx.enter_context(tc.tile_pool(name="consts", bufs=1))
                    ident, _, _, ones1 = load_consts(cp)
                    w_sb = load_w(cp, w_d, "wN")
                    bN_sb = cp.tile([1, 2048], F32, tag="bN")
                    nc.sync.dma_start(out=bN_sb[:], in_=b_d)
                    _precompute_pass(
                        nc, tc, ctx,
                        srcs=[(o_prev[:], None, 4)],
                        dsts=[(xw_next[0:T], bN_sb, [w_sb])],
                        ident=ident, ones1=ones1)

    nc.compile()
    return nc


# gate order: keras (i, f, g, o) -> ours (f, i, o, g)
def _perm_cols(W):
    return np.concatenate(
        [W[..., 512:1024], W[..., 0:512], W[..., 1536:2048], W[..., 1024:1536]],
        axis=-1)


def _prep_w(W):
    """[K, 2048] -> [128, (K/128)*2048], k-chunk-major in the free dim."""
    Wp = _perm_cols(np.asarray(W, np.float32))
    K = Wp.shape[0]
    return np.ascontiguousarray(
        Wp.reshape(K // 128, 128, 2048).transpose(1, 0, 2).reshape(128, -1))


_CACHED = {}


def _get_nc(T_):
    if T_ not in _CACHED:
        _CACHED[T_] = _build(T_)
    return _CACHED[T_]


def kernel(input_seq, hidden, emb_table,
           W1f, U1f, b1f, W1b, U1b, b1b,
           W2, U2, b2, W3, U3, b3, W4, U4, b4):
    T_ = input_seq.shape[1]
    nc = _get_nc(T_)

    ident = np.eye(128, dtype=np.float32)
    sp_fio = np.zeros((32, 72), np.float32)
    for g in range(3):
        for b_ in range(8):
            sp_fio[8 * g + b_, 32 * g + b_] = 1.0
    sp_g = np.zeros((32, 8), np.float32)
    for b_ in range(8):
        sp_g[24 + b_, b_] = 1.0
    ones1 = np.ones((1, 128), np.float32)

    W2_ = np.asarray(W2, np.float32)
    shared = {
        "emb": np.asarray(emb_table, np.float32),
        "ident": ident, "sp_fio": sp_fio, "sp_g": sp_g, "ones1": ones1,
        "w1f": _prep_w(W1f), "w1b": _prep_w(W1b),
        "w2f": _prep_w(W2_[0:512]), "w2b": _prep_w(W2_[512:1024]),
        "w3": _prep_w(W3), "w4": _prep_w(W4),
        "u1f": _prep_w(U1f), "u1b": _prep_w(U1b),
        "u2": _prep_w(U2), "u3": _prep_w(U3), "u4": _prep_w(U4),
        "b1f": _perm_cols(np.asarray(b1f, np.float32))[None, :],
        "b1b": _perm_cols(np.asarray(b1b, np.float32))[None, :],
        "b2": _perm_cols(np.asarray(b2, np.float32))[None, :],
        "b3": _perm_cols(np.asarray(b3, np.float32))[None, :],
        "b4": _perm_cols(np.asarray(b4, np.float32))[None, :],
    }
    seq = np.asarray(input_seq).astype(np.int32)
    in_maps = []
    for c in range(NCORES):
        m = dict(shared)
        m["idx"] = np.ascontiguousarray(
            seq[BL * c:BL * (c + 1)].T[:, :, None])  # [T, BL, 1]
        in_maps.append(m)

    res = run_bass_kernel_spmd(nc, in_maps, list(range(NCORES)))

    out = np.empty((B, T_, UNITS), np.float32)
    c_state = np.empty((B, UNITS), np.float32)
    for c in range(NCORES):
        r = res.results[c]
        out[BL * c:BL * (c + 1)] = r["out4"].transpose(1, 0, 2)
        c_state[BL * c:BL * (c + 1)] = r["c4"]
    h_state = np.ascontiguousarray(out[:, -1, :])
    return out, h_state, c_state


if __name__ == "__main__":
    pass
